# revision 1
# baseline (speedup 1.0000x reference)
"""BaiChuan attention block (QKV proj + RoPE + causal attention + o_proj) on 8 NeuronCores.

Sharding: tensor-parallel over heads. Each core owns 4 of the 32 heads:
W_pack columns (q/k/v slices) are column-sharded, w_o is row-sharded, and the
8 partial o_proj outputs are summed on the host (cheap f32 reduce) instead of
an on-device all-reduce.

Everything on-device runs in bf16 (fp32 PSUM accumulation). Activations are
kept feature-major ("transposed", [feature, batch*seq]) end to end so that
softmax runs along the PSUM partition axis and no probability-tile transposes
are needed:
  scoresT[k, q] = K_chunk @ Q_group   (lhsT = KT chunk, rhs = QT group)
  probsT = exp(scoresT)               (softmax scale pre-folded into Q rope tables,
                                       no max subtraction: |scores| <= ~12 for this
                                       distribution so exp is safe in fp32/bf16)
  causal mask  = sliding slice of a constant 0/1 tile, applied only to the 4
                 diagonal chunks of each 512-wide q group
  outT[d, q]  += V_kd chunk @ probsT  (PSUM accumulate over k chunks)
  sums[1, q]  += ones @ probsT        (softmax denominator via 1-row matmul)
  normalize: reciprocal_approx_fast on VectorE (ScalarE stays a pure-Exp stream:
             mixing activation functions forces 1.3us ACT table reloads),
             partition-broadcast via a K=1 outer-product matmul, multiplied into
             outT on the PSUM->SBUF copy.

Scheduling for the HAM clock gate (engine streams are static and in-order, so
overlap must be baked into emission order):
  - qkv projection first: resident weights, double-buffered activations, input
    DMAs split across both HWDGE queues; constants go on the gpsimd SWDGE queue.
  - attention runs q-group-outer / head-inner per batch, and o_proj m-chunks are
    hand-interleaved one-per-j into the attention j-loops as soon as their seq
    tile's four head slices exist. o_proj reads the normalized stage tiles
    straight from SBUF (no DRAM bounce), so exp (ScalarE-bound) hides under
    o_proj matmuls and TensorE never idles long enough to re-throttle.
"""

import os
from collections import deque
import numpy as np
import ml_dtypes

import concourse.bass as bass
import concourse.tile as tile
import concourse.mybir as mybir
from concourse import bacc
from concourse.bass_utils import run_bass_kernel_spmd

F32 = mybir.dt.float32
BF16 = mybir.dt.bfloat16
AF = mybir.ActivationFunctionType
BF = ml_dtypes.bfloat16

B, S, H = 2, 2048, 4096
BS = B * S                      # 4096 tokens
D = 128                         # head dim
NCORES = 8
NH_LOC = 4                      # heads per core (32 / 8)
HK = H // 128                   # 32 contraction chunks for qkv proj
M_QKV = 3 * NH_LOC              # 12 qkv output row-chunks per core
ST = 512                        # seq tile
NT = BS // ST                   # 8 seq tiles
GP = S // ST                    # 4 q-groups per sequence
ROPE_THETA = 10000.0
SCALE = D ** -0.5

LAST_RESULT = None              # BassKernelResults of the most recent run (for test.py)


def _build_program():
    nc = bacc.Bacc()

    hT = nc.dram_tensor("hT", [H, BS], BF16, kind="ExternalInput")
    w1 = nc.dram_tensor("w1", [H, M_QKV * 128], BF16, kind="ExternalInput")
    wo = nc.dram_tensor("wo", [NH_LOC * 128, H], BF16, kind="ExternalInput")
    cq = nc.dram_tensor("cq", [128, S], BF16, kind="ExternalInput")
    sq = nc.dram_tensor("sq", [128, S], BF16, kind="ExternalInput")
    ck = nc.dram_tensor("ck", [128, S], BF16, kind="ExternalInput")
    sk = nc.dram_tensor("sk", [128, S], BF16, kind="ExternalInput")
    maskd = nc.dram_tensor("mask", [128, 384 + ST], BF16, kind="ExternalInput")
    out = nc.dram_tensor("out", [H, BS], BF16, kind="ExternalOutput")

    with tile.TileContext(nc) as tc:
        with (
            tc.tile_pool(name="cons", bufs=1) as cons,
            tc.tile_pool(name="dram", bufs=1, space="DRAM") as dram,
            tc.tile_pool(name="ps_acc", bufs=3, space="PSUM") as ps_acc,
            tc.tile_pool(name="ps_sc", bufs=3, space="PSUM") as ps_sc_p,
            tc.tile_pool(name="ps_sum", bufs=2, space="PSUM") as ps_sum_p,
        ):
            # long-lived constants (gpsimd SWDGE queue: off the critical-path
            # HWDGE queues that feed the first projection matmuls)
            cq_sb = cons.tile([128, S], BF16, tag="cq")
            nc.gpsimd.dma_start(cq_sb[:], cq[:])
            sq_sb = cons.tile([128, S], BF16, tag="sq")
            nc.gpsimd.dma_start(sq_sb[:], sq[:])
            ck_sb = cons.tile([128, S], BF16, tag="ck")
            nc.gpsimd.dma_start(ck_sb[:], ck[:])
            sk_sb = cons.tile([128, S], BF16, tag="sk")
            nc.gpsimd.dma_start(sk_sb[:], sk[:])
            mask_sb = cons.tile([128, 384 + ST], BF16, tag="mask")
            nc.gpsimd.dma_start(mask_sb[:], maskd[:])
            ones_col = cons.tile([128, 1], BF16, tag="ones_col")
            nc.vector.memset(ones_col[:], 1.0)
            ones_row = cons.tile([1, 128], BF16, tag="ones_row")
            nc.vector.memset(ones_row[:], 1.0)

            qkv_dram = dram.tile([M_QKV * 128, BS], BF16)

            hT3 = hT.rearrange("(ko p) s -> p ko s", p=128)
            w13 = w1.rearrange("(ko p) m -> p ko m", p=128)

            # ---------------- Phase 1: qkvT = w1.T @ hT ----------------
            with (
                tc.tile_pool(name="w1p", bufs=1) as w1p,
                tc.tile_pool(name="htp", bufs=2) as htp,
                tc.tile_pool(name="p1o", bufs=3) as p1o,
            ):
                w_sb = w1p.tile([128, HK, M_QKV * 128], BF16, tag="w1")
                for m in range(M_QKV):
                    nc.scalar.dma_start(
                        w_sb[:, :, m * 128:(m + 1) * 128],
                        w13[:, :, m * 128:(m + 1) * 128])
                for t in range(NT):
                    ht = htp.tile([128, HK, ST], BF16, tag="ht")
                    for oct_ in range(4):
                        nc.sync.dma_start(
                            ht[:, oct_ * 8:(oct_ + 1) * 8],
                            hT3[:, oct_ * 8:(oct_ + 1) * 8, t * ST:(t + 1) * ST])
                    for m in range(M_QKV):
                        ps = ps_acc.tile([128, ST], F32, tag="acc")
                        for ko in range(HK):
                            nc.tensor.matmul(
                                ps[:], w_sb[:, ko, m * 128:(m + 1) * 128],
                                ht[:, ko], start=(ko == 0), stop=(ko == HK - 1))
                        ob = p1o.tile([128, ST], BF16, tag="ob")
                        nc.vector.tensor_copy(ob[:], ps[:])
                        nc.scalar.dma_start(
                            qkv_dram[m * 128:(m + 1) * 128, t * ST:(t + 1) * ST], ob[:])

            # ---------------- Phase 2+3: attention with interleaved o_proj ----------------
            with (
                tc.tile_pool(name="xload", bufs=2) as xload,
                tc.tile_pool(name="headp", bufs=5) as headp,
                tc.tile_pool(name="probsp", bufs=6) as probsp,
                tc.tile_pool(name="stagep", bufs=12) as stagep,
                tc.tile_pool(name="miscp", bufs=2) as miscp,
                tc.tile_pool(name="p3w", bufs=1) as wop,
                tc.tile_pool(name="p3o", bufs=4) as p3o,
            ):
                # o_proj weights load early, overlapping the whole attention phase
                wo_sb = wop.tile([128, NH_LOC, H], BF16, tag="wo")
                nc.scalar.dma_start(wo_sb[:], wo.rearrange("(ko p) f -> p ko f", p=128))

                # FIFO of pending o_proj m-chunk emitters, popped one per attention j
                filler = deque()

                def emit_filler():
                    if filler:
                        filler.popleft()()

                def make_oproj_chunk(t, m, stages):
                    def emit():
                        ps = ps_acc.tile([128, ST], F32, tag="acc", name=f"ps_o_{t}_{m}")
                        for ko in range(NH_LOC):
                            nc.tensor.matmul(
                                ps[:], wo_sb[:, ko, m * 128:(m + 1) * 128],
                                stages[ko][:],
                                start=(ko == 0), stop=(ko == NH_LOC - 1))
                        ob = p3o.tile([128, ST], BF16, tag="ob3", name=f"ob3_{t}_{m}")
                        nc.vector.tensor_copy(ob[:], ps[:])
                        nc.sync.dma_start(
                            out[m * 128:(m + 1) * 128, t * ST:(t + 1) * ST], ob[:])
                    return emit

                def load_and_rope(b, h):
                    q_rows = h * 128
                    k_rows = (NH_LOC + h) * 128
                    v_rows = (2 * NH_LOC + h) * 128
                    cols = slice(b * S, (b + 1) * S)

                    xq = xload.tile([128, S], BF16, tag="xq")
                    nc.sync.dma_start(xq[:], qkv_dram[q_rows:q_rows + 128, cols])
                    xqs = xload.tile([128, S], BF16, tag="xqs")
                    nc.sync.dma_start(xqs[0:64, :], qkv_dram[q_rows + 64:q_rows + 128, cols])
                    nc.sync.dma_start(xqs[64:128, :], qkv_dram[q_rows:q_rows + 64, cols])
                    xk = xload.tile([128, S], BF16, tag="xk")
                    nc.sync.dma_start(xk[:], qkv_dram[k_rows:k_rows + 128, cols])
                    xks = xload.tile([128, S], BF16, tag="xks")
                    nc.sync.dma_start(xks[0:64, :], qkv_dram[k_rows + 64:k_rows + 128, cols])
                    nc.sync.dma_start(xks[64:128, :], qkv_dram[k_rows:k_rows + 64, cols])

                    qt = headp.tile([128, S], BF16, tag="qt")
                    tmp = miscp.tile([128, S], BF16, tag="ropetmp")
                    nc.vector.tensor_mul(qt[:], xq[:], cq_sb[:])
                    nc.vector.tensor_mul(tmp[:], xqs[:], sq_sb[:])
                    nc.vector.tensor_add(qt[:], qt[:], tmp[:])
                    kt = headp.tile([128, S], BF16, tag="kt")
                    tmp2 = miscp.tile([128, S], BF16, tag="ropetmp")
                    nc.vector.tensor_mul(kt[:], xk[:], ck_sb[:])
                    nc.vector.tensor_mul(tmp2[:], xks[:], sk_sb[:])
                    nc.vector.tensor_add(kt[:], kt[:], tmp2[:])

                    v_kd = headp.tile([128, S // 128, 128], BF16, tag="vkd")
                    nc.sync.dma_start_transpose(v_kd[:], qkv_dram[v_rows:v_rows + 128, cols])
                    return qt, kt, v_kd

                def attention_group(b, h, g, qt, kt, v_kd):
                    q0 = g * ST
                    nj = 4 * g + 4
                    ps_out = ps_acc.tile([128, ST], F32, tag="acc", name=f"ps_out_{b}_{h}_{g}")
                    ps_sum = ps_sum_p.tile([1, ST], F32, tag="sum", name=f"ps_sum_{b}_{h}_{g}")
                    for j in range(nj):
                        ps_sc = ps_sc_p.tile([128, ST], F32, tag="sc", name=f"ps_sc_{b}_{h}_{g}_{j}")
                        nc.tensor.matmul(ps_sc[:], kt[:, j * 128:(j + 1) * 128],
                                         qt[:, q0:q0 + ST], start=True, stop=True)
                        probs = probsp.tile([128, ST], BF16, tag="probs", name=f"probs_{b}_{h}_{g}_{j}")
                        nc.scalar.activation(probs[:], ps_sc[:], AF.Exp)
                        if j >= 4 * g:
                            r = (j - 4 * g) * 128
                            nc.vector.tensor_mul(
                                probs[:], probs[:], mask_sb[:, 384 - r:384 - r + ST])
                        nc.tensor.matmul(ps_out[:], v_kd[:, j], probs[:],
                                         start=(j == 0), stop=(j == nj - 1))
                        nc.tensor.matmul(ps_sum[:], ones_col[:], probs[:],
                                         start=(j == 0), stop=(j == nj - 1))
                        emit_filler()
                    # normalize (VectorE; ScalarE stays pure-Exp to avoid table reloads)
                    rec32 = miscp.tile([1, ST], F32, tag="rec32")
                    nc.vector.reciprocal_approx_fast(rec32[:], ps_sum[:])
                    recip = miscp.tile([1, ST], BF16, tag="recip")
                    nc.vector.tensor_copy(recip[:], rec32[:])
                    ps_bc = ps_sc_p.tile([128, ST], F32, tag="sc", name=f"ps_bc_{b}_{h}_{g}")
                    nc.tensor.matmul(ps_bc[:], ones_row[:], recip[:],
                                     start=True, stop=True)
                    bc_sb = miscp.tile([128, ST], F32, tag="bc_sb")
                    nc.vector.tensor_copy(bc_sb[:], ps_bc[:])
                    stage = stagep.tile([128, ST], BF16, tag="stage", name=f"stage_{b}_{h}_{g}")
                    nc.vector.tensor_mul(stage[:], ps_out[:], bc_sb[:])
                    return stage

                for b in range(B):
                    tiles = [load_and_rope(b, h) for h in range(NH_LOC)]
                    for g in range(GP):
                        stages = []
                        for h in range(NH_LOC):
                            qt, kt, v_kd = tiles[h]
                            stages.append(attention_group(b, h, g, qt, kt, v_kd))
                        t = b * GP + g
                        for m in range(H // 128):
                            filler.append(make_oproj_chunk(t, m, stages))
                while filler:
                    filler.popleft()()

    nc.finalize()
    return nc


def _prep_inputs(positions, hidden_states, w_pack, w_o):
    pos = np.asarray(positions).astype(np.float32)
    hid = np.asarray(hidden_states, dtype=np.float32)
    w_pack = np.asarray(w_pack, dtype=np.float32)
    w_o = np.asarray(w_o, dtype=np.float32)

    hT = np.ascontiguousarray(hid.reshape(BS, H).T).astype(BF)

    inv_freq = 1.0 / (ROPE_THETA ** (np.arange(0, D, 2, dtype=np.float32) / D))
    ang = pos[None, :] * inv_freq[:, None]              # [64, S]
    cos = np.cos(ang).astype(np.float32)
    sin = np.sin(ang).astype(np.float32)
    cos_t = np.concatenate([cos, cos], 0)               # [128, S]
    sinS_t = np.concatenate([-sin, sin], 0)
    cq = np.ascontiguousarray(cos_t * SCALE).astype(BF)
    sq = np.ascontiguousarray(sinS_t * SCALE).astype(BF)
    ck = np.ascontiguousarray(cos_t).astype(BF)
    sk = np.ascontiguousarray(sinS_t).astype(BF)

    mask = (np.arange(384 + ST)[None, :] >= (np.arange(128)[:, None] + 384)).astype(BF)

    in_maps = []
    for c in range(NCORES):
        j0 = 512 * c
        w1 = np.concatenate([w_pack[:, j0:j0 + 512],
                             w_pack[:, H + j0:H + j0 + 512],
                             w_pack[:, 2 * H + j0:2 * H + j0 + 512]], axis=1).astype(BF)
        wo = np.ascontiguousarray(w_o[j0:j0 + 512, :]).astype(BF)
        in_maps.append({
            "hT": hT, "w1": np.ascontiguousarray(w1), "wo": wo,
            "cq": cq, "sq": sq, "ck": ck, "sk": sk, "mask": mask,
        })
    return in_maps


def kernel(positions, hidden_states, w_pack, w_o):
    global LAST_RESULT
    nc = _build_program()
    in_maps = _prep_inputs(positions, hidden_states, w_pack, w_o)
    res = run_bass_kernel_spmd(
        nc, in_maps, core_ids=list(range(NCORES)),
        trace=bool(os.environ.get("BASS_TRACE")))
    LAST_RESULT = res
    acc = np.zeros((H, BS), np.float32)
    for r in res.results:
        acc += r["out"].astype(np.float32)
    return np.ascontiguousarray(acc.T).reshape(B, S, H)



# revision 4
# speedup vs baseline: 1.0984x; 1.0984x over previous
"""BaiChuan attention block (QKV proj + RoPE + causal attention + o_proj) on 8 NeuronCores.

Sharding: tensor-parallel over heads. Each core owns 4 of the 32 heads:
W_pack columns (q/k/v slices) are column-sharded, w_o is row-sharded, and the
8 partial o_proj outputs are summed on the host (cheap f32 reduce).

Restructured vs the DRAM-bounce baseline:
  - qkv stays RESIDENT in SBUF (no DRAM round trip): phase-1 writes q/k as
    [d, s] head tiles and v DIRECTLY TRANSPOSED as [s, d] chunk tiles by
    swapping matmul operands (lhsT = hT chunk, rhs = w1_v), so attention needs
    no dma_start_transpose and no reload.
  - batch-serial schedule: S1 qkv(b0) | S2 attn(b0)+o_proj(b0) | S3 qkv(b1)
    | S4 attn(b1)+o_proj(b1). o_proj chunk emitters are popped one-per-j as
    TensorE fillers inside the attention j-loops.
  - attention j-loop software-pipelines the score matmul one chunk ahead
    (emission order SC(j+1), PV(j), SUM(j), filler), so the ScalarE exp
    latency for chunk j hides under ~1.5us of other TensorE work.
  - causal diagonal chunks are N-TRIMMED: score/exp/PV/SUM only cover the
    valid q-range [128r, 512), and the mask multiply shrinks to a [128,128]
    triangle.
  - softmax scale is folded into the exp (ACT free affine), so q and k share
    one unscaled cos/sin table pair.
  - the reciprocal row -> 128-partition broadcast runs on idle GpSimdE
    (partition_broadcast) instead of a K=1 TensorE matmul.
  - RoPE rotate-half copies are SBUF->SBUF DMAs; rope multiplies run in-place
    on the resident q/k tiles, emission-staggered so the rot DMA completes
    before VectorE needs it.
"""

import os
from collections import deque
import numpy as np
import ml_dtypes

import concourse.bass as bass
import concourse.tile as tile
import concourse.mybir as mybir
from concourse import bacc
from concourse.bass_utils import run_bass_kernel_spmd

F32 = mybir.dt.float32
BF16 = mybir.dt.bfloat16
AF = mybir.ActivationFunctionType
BF = ml_dtypes.bfloat16

B, S, H = 2, 2048, 4096
BS = B * S                      # 4096 tokens
D = 128                         # head dim
NCORES = 8
NH_LOC = 4                      # heads per core (32 / 8)
HK = H // 128                   # 32 contraction chunks
ST = 512                        # seq tile / q-group width
GP = S // ST                    # 4 q-groups per batch
ROPE_THETA = 10000.0
SCALE = D ** -0.5

LAST_RESULT = None              # BassKernelResults of the most recent run (for test.py)


def _build_program():
    nc = bacc.Bacc()

    hT = nc.dram_tensor("hT", [H, BS], BF16, kind="ExternalInput")
    w1 = nc.dram_tensor("w1", [H, 8 * 128], BF16, kind="ExternalInput")   # q|k heads
    w1v = nc.dram_tensor("w1v", [H, 512], BF16, kind="ExternalInput")     # v cols
    wo = nc.dram_tensor("wo", [NH_LOC * 128, H], BF16, kind="ExternalInput")
    cs = nc.dram_tensor("cs", [128, S], BF16, kind="ExternalInput")
    sn = nc.dram_tensor("sn", [128, S], BF16, kind="ExternalInput")
    maskd = nc.dram_tensor("mask", [128, 128], BF16, kind="ExternalInput")
    out = nc.dram_tensor("out", [H, BS], BF16, kind="ExternalOutput")

    hT3 = hT.rearrange("(ko p) s -> p ko s", p=128)
    w13 = w1.rearrange("(ko p) m -> p ko m", p=128)
    w1v3 = w1v.rearrange("(ko p) m -> p ko m", p=128)
    wo3 = wo.rearrange("(ko p) f -> p ko f", p=128)

    with tile.TileContext(nc) as tc:
        with (
            tc.tile_pool(name="cons", bufs=1) as cons,
            tc.tile_pool(name="htp", bufs=3) as htp,
            tc.tile_pool(name="w1p", bufs=2) as w1p,
            tc.tile_pool(name="qkp", bufs=8) as qkp,
            tc.tile_pool(name="vp", bufs=16) as vp,
            tc.tile_pool(name="rotp", bufs=1) as rotp,
            tc.tile_pool(name="probsp", bufs=3) as probsp,
            tc.tile_pool(name="stagep", bufs=8) as stagep,
            tc.tile_pool(name="obp", bufs=2) as obp,
            tc.tile_pool(name="miscp", bufs=1) as miscp,
            tc.tile_pool(name="wop", bufs=1) as wop,
            tc.tile_pool(name="ps_acc", bufs=2, space="PSUM") as ps_acc,
            tc.tile_pool(name="ps_sc", bufs=2, space="PSUM") as ps_scp,
            tc.tile_pool(name="ps_out", bufs=2, space="PSUM") as ps_outp,
            tc.tile_pool(name="ps_sum", bufs=2, space="PSUM") as ps_sump,
        ):
            # ---- constants (gpsimd SWDGE queue: off the hot HWDGE queues) ----
            cs_sb = cons.tile([128, S], BF16, tag="cs")
            nc.gpsimd.dma_start(cs_sb[:], cs[:])
            sn_sb = cons.tile([128, S], BF16, tag="sn")
            nc.gpsimd.dma_start(sn_sb[:], sn[:])
            mask_sb = cons.tile([128, 128], BF16, tag="mask")
            nc.gpsimd.dma_start(mask_sb[:], maskd[:])
            ones_col = cons.tile([128, 1], BF16, tag="ones_col")
            nc.vector.memset(ones_col[:], 1.0)

            # v-part of w_pack: resident whole kernel (moving operand of v matmuls)
            w1v_sb = w1p.tile([128, HK, 512], BF16, tag="w1v", bufs=1)
            nc.scalar.dma_start(w1v_sb[:], w1v3[:])
            # o_proj weights (SWDGE; needed from S2 on)
            wo_sb = wop.tile([128, NH_LOC, H], BF16, tag="wo")
            nc.gpsimd.dma_start(wo_sb[:], wo3[:])

            filler = deque()

            def emit_filler():
                if filler:
                    filler.popleft()()

            # ---------------- phase 1 (per batch): qkv projection ----------------
            def emit_phase1(b, qk_tiles, v_tiles):
                for tl in range(4):
                    t = 4 * b + tl
                    fwd = (tl % 2 == 0)
                    halves = (0, 1) if fwd else (1, 0)
                    ht_tiles = {}
                    for hv in halves:
                        htt = htp.tile([128, 16, ST], BF16, tag="ht",
                                       name=f"ht_{t}_{hv}")
                        for oc in range(2):
                            nc.sync.dma_start(
                                htt[:, oc * 8:(oc + 1) * 8],
                                hT3[:, hv * 16 + oc * 8: hv * 16 + (oc + 1) * 8,
                                    t * ST:(t + 1) * ST])
                        ht_tiles[hv] = htt
                    ko_order = list(range(HK)) if fwd else list(range(HK - 1, -1, -1))

                    for m in range(8):          # q heads 0-3, k heads 0-3
                        w1c = w1p.tile([128, HK, 128], BF16, tag="w1c",
                                       name=f"w1c_{t}_{m}")
                        nc.scalar.dma_start(w1c[:], w13[:, :, m * 128:(m + 1) * 128])
                        ps = ps_acc.tile([128, ST], F32, tag="acc",
                                         name=f"ps_p1_{t}_{m}")
                        for i, ko in enumerate(ko_order):
                            nc.tensor.matmul(
                                ps[:], w1c[:, ko], ht_tiles[ko // 16][:, ko % 16],
                                start=(i == 0), stop=(i == HK - 1))
                        kind = 'q' if m < 4 else 'k'
                        dst = qk_tiles[(kind, m % 4)]
                        nc.vector.tensor_copy(dst[:, tl * ST:(tl + 1) * ST], ps[:])
                        if tl == 3:
                            # stagger rope emission so rot DMAs land before DVE use
                            _rot_dma(b, m, qk_tiles)
                            if m >= 1:
                                _rope_dve(b, m - 1, qk_tiles)

                    for sc in range(4):         # v, directly transposed: [s, d]
                        ps = ps_acc.tile([128, ST], F32, tag="acc",
                                         name=f"ps_v_{t}_{sc}")
                        for i, ko in enumerate(ko_order):
                            nc.tensor.matmul(
                                ps[:],
                                ht_tiles[ko // 16][:, ko % 16, sc * 128:(sc + 1) * 128],
                                w1v_sb[:, ko],
                                start=(i == 0), stop=(i == HK - 1))
                        nc.scalar.copy(v_tiles[4 * tl + sc][:], ps[:])
                    if tl == 3:
                        _rope_dve(b, 7, qk_tiles)

            rot_tiles = {}

            def _rot_dma(b, m, qk_tiles):
                kind = 'q' if m < 4 else 'k'
                x = qk_tiles[(kind, m % 4)]
                rot = rotp.tile([128, S], BF16, tag="rot", name=f"rot_{b}_{m}")
                nc.gpsimd.dma_start(rot[0:64, :], x[64:128, :])
                nc.gpsimd.dma_start(rot[64:128, :], x[0:64, :])
                rot_tiles[(b, m)] = rot

            def _rope_dve(b, m, qk_tiles):
                kind = 'q' if m < 4 else 'k'
                x = qk_tiles[(kind, m % 4)]
                rot = rot_tiles.pop((b, m))
                nc.vector.tensor_mul(rot[:], rot[:], sn_sb[:])
                nc.vector.tensor_mul(x[:], x[:], cs_sb[:])
                nc.vector.tensor_add(x[:], x[:], rot[:])

            # ---------------- attention (per batch, group, head) ----------------
            def emit_attn(b, g, h, qk_tiles, v_tiles):
                nj = 4 * g + 4
                q0 = g * ST
                qt = qk_tiles[('q', h)]
                kt = qk_tiles[('k', h)]
                ps_out = ps_outp.tile([128, ST], F32, tag="out",
                                      name=f"ps_out_{b}_{g}_{h}")
                ps_sum = ps_sump.tile([1, ST], F32, tag="sum",
                                      name=f"ps_sum_{b}_{g}_{h}")
                sc_tiles = {}

                def emit_sc(j):
                    r = j - 4 * g
                    c0 = 128 * r if r > 0 else 0
                    ps_sc = ps_scp.tile([128, ST], F32, tag="sc",
                                        name=f"ps_sc_{b}_{g}_{h}_{j}")
                    nc.tensor.matmul(ps_sc[:, c0:], kt[:, j * 128:(j + 1) * 128],
                                     qt[:, q0 + c0:q0 + ST], start=True, stop=True)
                    sc_tiles[j] = (ps_sc, c0)

                emit_sc(0)
                for j in range(nj):
                    if j + 1 < nj:
                        emit_sc(j + 1)
                    ps_sc, c0 = sc_tiles.pop(j)
                    probs = probsp.tile([128, ST], BF16, tag="probs",
                                        name=f"probs_{b}_{g}_{h}_{j}")
                    nc.scalar.activation(probs[:, c0:], ps_sc[:, c0:], AF.Exp,
                                         scale=SCALE)
                    if j - 4 * g >= 0:
                        nc.vector.tensor_mul(probs[:, c0:c0 + 128],
                                             probs[:, c0:c0 + 128], mask_sb[:])
                    nc.tensor.matmul(ps_out[:, c0:],
                                     v_tiles[j][:, h * 128:(h + 1) * 128],
                                     probs[:, c0:],
                                     start=(j == 0), stop=(j == nj - 1))
                    nc.tensor.matmul(ps_sum[:, c0:], ones_col[:], probs[:, c0:],
                                     start=(j == 0), stop=(j == nj - 1))
                    emit_filler()

                rec32 = miscp.tile([1, ST], F32, tag="rec32",
                                   name=f"rec32_{b}_{g}_{h}")
                nc.vector.reciprocal_approx_fast(rec32[:], ps_sum[:])
                rec16 = miscp.tile([1, ST], BF16, tag="rec16",
                                   name=f"rec16_{b}_{g}_{h}")
                nc.vector.tensor_copy(rec16[:], rec32[:])
                rbc = miscp.tile([128, ST], BF16, tag="rbc", bufs=2,
                                 name=f"rbc_{b}_{g}_{h}")
                nc.gpsimd.partition_broadcast(rbc[:], rec16[:], channels=128)
                stage = stagep.tile([128, ST], BF16, tag="stage",
                                    name=f"stage_{b}_{g}_{h}")
                nc.vector.tensor_mul(stage[:], ps_out[:], rbc[:])
                return stage

            # ---------------- o_proj chunk emitters (TensorE fillers) ----------------
            def make_oproj(t, m, stages):
                def emit():
                    ps = ps_acc.tile([128, ST], F32, tag="acc", name=f"ps_o_{t}_{m}")
                    for ko in range(NH_LOC):
                        nc.tensor.matmul(ps[:], wo_sb[:, ko, m * 128:(m + 1) * 128],
                                         stages[ko][:],
                                         start=(ko == 0), stop=(ko == NH_LOC - 1))
                    ob = obp.tile([128, ST], BF16, tag="ob", name=f"ob_{t}_{m}")
                    if m % 2 == 0:
                        nc.vector.tensor_copy(ob[:], ps[:])
                        nc.sync.dma_start(
                            out[m * 128:(m + 1) * 128, t * ST:(t + 1) * ST], ob[:])
                    else:
                        nc.scalar.copy(ob[:], ps[:])
                        nc.scalar.dma_start(
                            out[m * 128:(m + 1) * 128, t * ST:(t + 1) * ST], ob[:])
                return emit

            # ---------------- schedule ----------------
            for b in range(B):
                qk_tiles = {}
                for kind in ('q', 'k'):
                    for h in range(NH_LOC):
                        qk_tiles[(kind, h)] = qkp.tile(
                            [128, S], BF16, tag="qk", name=f"{kind}{h}_b{b}")
                v_tiles = [vp.tile([128, ST], BF16, tag="v", name=f"v{i}_b{b}")
                           for i in range(16)]
                emit_phase1(b, qk_tiles, v_tiles)
                for g in range(GP):
                    stages = [emit_attn(b, g, h, qk_tiles, v_tiles)
                              for h in range(NH_LOC)]
                    t = 4 * b + g
                    for m in range(H // 128):
                        filler.append(make_oproj(t, m, stages))
            while filler:
                filler.popleft()()

    nc.finalize()
    return nc


def _prep_inputs(positions, hidden_states, w_pack, w_o):
    pos = np.asarray(positions).astype(np.float32)
    hid = np.asarray(hidden_states, dtype=np.float32)
    w_pack = np.asarray(w_pack, dtype=np.float32)
    w_o = np.asarray(w_o, dtype=np.float32)

    hT = np.ascontiguousarray(hid.reshape(BS, H).T).astype(BF)

    inv_freq = 1.0 / (ROPE_THETA ** (np.arange(0, D, 2, dtype=np.float32) / D))
    ang = pos[None, :] * inv_freq[:, None]              # [64, S]
    cos = np.cos(ang).astype(np.float32)
    sin = np.sin(ang).astype(np.float32)
    cs = np.ascontiguousarray(np.concatenate([cos, cos], 0)).astype(BF)   # [128, S]
    sn = np.ascontiguousarray(np.concatenate([-sin, sin], 0)).astype(BF)

    # [128, 128] lower-triangle-inclusive: mask[k, q] = 1 if q >= k
    mask = (np.arange(128)[None, :] >= np.arange(128)[:, None]).astype(BF)

    in_maps = []
    for c in range(NCORES):
        j0 = 512 * c
        w1 = np.concatenate([w_pack[:, j0:j0 + 512],
                             w_pack[:, H + j0:H + j0 + 512]], axis=1).astype(BF)
        w1v = np.ascontiguousarray(w_pack[:, 2 * H + j0:2 * H + j0 + 512]).astype(BF)
        wo = np.ascontiguousarray(w_o[j0:j0 + 512, :]).astype(BF)
        in_maps.append({
            "hT": hT, "w1": np.ascontiguousarray(w1), "w1v": w1v, "wo": wo,
            "cs": cs, "sn": sn, "mask": mask,
        })
    return in_maps


def kernel(positions, hidden_states, w_pack, w_o):
    global LAST_RESULT
    nc = _build_program()
    in_maps = _prep_inputs(positions, hidden_states, w_pack, w_o)
    res = run_bass_kernel_spmd(
        nc, in_maps, core_ids=list(range(NCORES)),
        trace=bool(os.environ.get("BASS_TRACE")))
    LAST_RESULT = res
    acc = np.zeros((H, BS), np.float32)
    for r in res.results:
        acc += r["out"].astype(np.float32)
    return np.ascontiguousarray(acc.T).reshape(B, S, H)
